# revision 4
# baseline (speedup 1.0000x reference)
"""GCN layer kernel for nn_GcnNet_17695265259748 on 8 Trainium2 NeuronCores.

out = A_norm @ mean_L(x) @ W + s*b, computed fully on-device:

  phase A  (per core, node-sharded): stream x shard, sum over L (fp16).
           W is pre-scaled by 1/L on host so the sum replaces the mean.
  phase B  AllGather of the per-core xm shards -> full node-feature table.
  phase C  edges partitioned by destination tile (128 dst nodes); source rows
           fetched with dma_gather (int16 indices; table split in two halves
           to fit int16), reduced per destination with a one-hot*norm
           selector matmul on the TensorEngine (PSUM f32 accumulate).
  phase D  project 128-wide aggregate through W (+ s*b bias) on TensorE.

Host does only integer edge bookkeeping (norms, grouping, index arrays).
Falls back to a pure numpy/scipy implementation if the device path fails.
"""

import os
import numpy as np

# problem constants
N, L, C, F = 50000, 20, 128, 300
E = 1600000
NCORES = 8

TRACE = bool(int(os.environ.get("GCN_TRACE", "0")))
FORCE_HOST = bool(int(os.environ.get("GCN_FORCE_HOST", "0")))
LAST_EXEC_NS = None
LAST_PROFILE = None

F16 = np.float16


class Cfg:
    def __init__(self, n=N, l=L, c=C, f=F, ncores=NCORES):
        assert n % ncores == 0
        self.N, self.L, self.C, self.F, self.NCORES = n, l, c, f, ncores
        self.NPC = n // ncores                       # real nodes per core
        self.NPN = -(-self.NPC // 128) * 128         # padded nodes per core
        self.NTILE = self.NPN // 128                 # dst tiles per core
        self.TROWS = self.NPN * ncores               # all-gathered table rows
        assert self.TROWS % 2 == 0
        self.HALF = self.TROWS // 2                  # int16 index split point
        assert self.HALF <= 32768


CFG = Cfg()


# ---------------------------------------------------------------------------
# host-side edge preprocessing
# ---------------------------------------------------------------------------

def _preprocess(cfg, edge_index):
    """Norms + destination-partitioned, half-split, padded slot layout."""
    n, npc, npn, half = cfg.N, cfg.NPC, cfg.NPN, cfg.HALF
    ntile, ncores = cfg.NTILE, cfg.NCORES
    ngrp = ntile * 2

    row = np.asarray(edge_index[0], dtype=np.int64)
    col = np.asarray(edge_index[1], dtype=np.int64)
    keep = row != col
    deg = np.bincount(row[keep], minlength=n).astype(np.float32) + 1.0
    dis = deg ** -0.5

    r = row[keep]
    c = col[keep]
    er = np.concatenate([r, np.arange(n, dtype=np.int64)])
    ec = np.concatenate([c, np.arange(n, dtype=np.int64)])
    ew = np.concatenate([(dis[r] * dis[c]).astype(np.float32),
                         (1.0 / deg).astype(np.float32)])

    # bias weight per destination (sum of norms incl. self loop)
    s = np.bincount(ec, weights=ew, minlength=n).astype(np.float32)

    trow = (er // npc) * npn + (er % npc)            # source row in AG table
    core = ec // npc                                 # owning (dst) core
    dloc = ec % npc                                  # local dst row
    dsub = dloc % 128                                # dst within its tile
    hbit = (trow >= half).astype(np.int64)
    gk = (dloc // 128) * 2 + hbit                    # group within core
    K = core * ngrp + gk

    counts = np.bincount(K, minlength=ncores * ngrp)
    caps = counts.reshape(ncores, ngrp).max(axis=0)
    caps = np.maximum(-(-caps // 128) * 128, 128).astype(np.int64)
    bases = np.concatenate([[0], np.cumsum(caps)])
    stot = int(bases[-1])

    order = np.argsort(K, kind="stable")
    Ks = K[order]
    gstart = np.concatenate([[0], np.cumsum(counts)])[:-1]
    rank = np.arange(len(K), dtype=np.int64) - gstart[Ks]
    slot = bases[Ks % ngrp] + rank
    core_s = Ks // ngrp

    idxa = np.zeros((ncores, stot), np.int16)
    dla = np.zeros((ncores, stot), F16)
    nma = np.zeros((ncores, stot), F16)
    idxa[core_s, slot] = (trow - hbit * half).astype(np.int16)[order]
    dla[core_s, slot] = dsub.astype(F16)[order]
    nma[core_s, slot] = ew.astype(F16)[order]

    # SBUF layouts: slot j -> idx[(j%16), j//16] (replicated x8 over 128
    # partitions); dloc/norm: slot j -> [j%128, j//128]
    gidx = idxa.reshape(ncores, stot // 16, 16).transpose(0, 2, 1)
    gidx = np.ascontiguousarray(np.tile(gidx, (1, 8, 1)))
    dl = np.ascontiguousarray(dla.reshape(ncores, stot // 128, 128).transpose(0, 2, 1))
    nm = np.ascontiguousarray(nma.reshape(ncores, stot // 128, 128).transpose(0, 2, 1))

    srow = np.zeros((ncores, npn), F16)
    srow_real = s.reshape(ncores, npc)
    srow[:, :npc] = srow_real.astype(F16)

    return caps, gidx, dl, nm, srow, s, deg, dis


# ---------------------------------------------------------------------------
# bass program
# ---------------------------------------------------------------------------

def _build_program(cfg, caps):
    import concourse.bass as bass
    import concourse.bacc as bacc
    import concourse.tile as tile
    from concourse import mybir

    f16 = mybir.dt.float16
    f32 = mybir.dt.float32
    i16 = mybir.dt.int16

    npn, ntile, trows, half = cfg.NPN, cfg.NTILE, cfg.TROWS, cfg.HALF
    l, c, f = cfg.L, cfg.C, cfg.F
    stot = int(np.sum(caps))
    bases = np.concatenate([[0], np.cumsum(caps)]).astype(np.int64)

    nc = bacc.Bacc("TRN2", target_bir_lowering=False, debug=False,
                   num_devices=cfg.NCORES)

    x_d = nc.dram_tensor("x", [npn, l * c], f16, kind="ExternalInput")
    gidx_d = nc.dram_tensor("gidx", [128, stot // 16], i16, kind="ExternalInput")
    dloc_d = nc.dram_tensor("dloc", [128, stot // 128], f16, kind="ExternalInput")
    norm_d = nc.dram_tensor("normv", [128, stot // 128], f16, kind="ExternalInput")
    srow_d = nc.dram_tensor("srow", [1, npn], f16, kind="ExternalInput")
    w_d = nc.dram_tensor("wmat", [c, f], f16, kind="ExternalInput")
    b_d = nc.dram_tensor("bvec", [1, f], f16, kind="ExternalInput")
    iota_d = nc.dram_tensor("iota2", [128, 128], f16, kind="ExternalInput")
    out_d = nc.dram_tensor("out", [npn, f], f16, kind="ExternalOutput")

    with tile.TileContext(nc) as tc:
        with (
            tc.tile_pool(name="const", bufs=1) as constp,
            tc.tile_pool(name="xin", bufs=3) as xpool,
            tc.tile_pool(name="red", bufs=3) as rpool,
            tc.tile_pool(name="xsum", bufs=1) as xsump,
            tc.tile_pool(name="gath", bufs=3) as gpool,
            tc.tile_pool(name="sel", bufs=3) as spool,
            tc.tile_pool(name="outs", bufs=3) as opool,
            tc.tile_pool(name="pagg", bufs=2, space="PSUM") as pagg,
            tc.tile_pool(name="pout", bufs=2, space="PSUM") as pout,
            tc.tile_pool(name="dram", bufs=1, space="DRAM") as dram,
        ):
            # resident constants
            gidx_sb = constp.tile([128, stot // 16], i16)
            dloc_sb = constp.tile([128, stot // 128], f16)
            norm_sb = constp.tile([128, stot // 128], f16)
            srow_sb = constp.tile([1, npn], f16)
            w_sb = constp.tile([c, f], f16)
            b_sb = constp.tile([1, f], f16)
            iota_sb = constp.tile([128, 128], f16)
            nc.sync.dma_start(gidx_sb[:], gidx_d[:])
            nc.sync.dma_start(dloc_sb[:], dloc_d[:])
            nc.sync.dma_start(norm_sb[:], norm_d[:])
            nc.sync.dma_start(srow_sb[:], srow_d[:])
            nc.sync.dma_start(w_sb[:], w_d[:])
            nc.sync.dma_start(b_sb[:], b_d[:])
            nc.sync.dma_start(iota_sb[:], iota_d[:])

            xm_my = dram.tile([npn, c], f16)
            xm_all = dram.tile([trows, c], f16)

            # ---- phase A: xsum over L --------------------------------------
            xsum_all = xsump.tile([128, ntile, c], f16)
            for t in range(ntile):
                xt = xpool.tile([128, l * c], f16)
                nc.sync.dma_start(xt[:], x_d[t * 128:(t + 1) * 128, :])
                hlc = (l // 2) * c          # 1280
                qlc = (l // 4) * c          # 640
                a1 = rpool.tile([128, hlc], f16, tag="a1")
                nc.vector.tensor_tensor(a1[:], xt[:, :hlc], xt[:, hlc:2 * hlc],
                                        mybir.AluOpType.add)
                a2 = rpool.tile([128, qlc], f16, tag="a2")
                nc.vector.tensor_tensor(a2[:], a1[:, :qlc], a1[:, qlc:2 * qlc],
                                        mybir.AluOpType.add)
                # remaining l//4 = 5 blocks of c
                a2v = a2[:].rearrange("p (k c) -> p k c", c=c)
                b1 = rpool.tile([128, c], f16, tag="b1")
                nc.vector.tensor_tensor(b1[:], a2v[:, 0, :], a2v[:, 1, :],
                                        mybir.AluOpType.add)
                b2 = rpool.tile([128, c], f16, tag="b2")
                nc.vector.tensor_tensor(b2[:], a2v[:, 2, :], a2v[:, 3, :],
                                        mybir.AluOpType.add)
                b3 = rpool.tile([128, c], f16, tag="b3")
                nc.vector.tensor_tensor(b3[:], b1[:], b2[:], mybir.AluOpType.add)
                nc.vector.tensor_tensor(xsum_all[:, t, :], b3[:], a2v[:, 4, :],
                                        mybir.AluOpType.add)

            # xsum_all[p, t, :] -> xm_my row t*128+p
            xmv = xm_my[:].rearrange("(t p) c -> p t c", p=128)
            nc.sync.dma_start(xmv, xsum_all[:])

            # ---- phase B: all-gather ---------------------------------------
            nc.gpsimd.collective_compute(
                "AllGather",
                mybir.AluOpType.bypass,
                replica_groups=[list(range(cfg.NCORES))],
                ins=[xm_my[:]],
                outs=[xm_all[:]],
            )

            # ---- phase C/D: per destination tile ---------------------------
            for d in range(ntile):
                pa = pagg.tile([128, 128], f32)
                nmm = 0
                total_mm = (caps[2 * d] + caps[2 * d + 1]) // 128
                for h in (0, 1):
                    cap = int(caps[2 * d + h])
                    kt = cap // 128
                    base = int(bases[2 * d + h])
                    gt = gpool.tile([128, kt * c], f16, tag="gt")
                    src = xm_all[h * half:(h + 1) * half, :]
                    nc.gpsimd.dma_gather(
                        gt[:].rearrange("p (k c) -> p k c", c=c),
                        src,
                        gidx_sb[:, base // 16:(base + cap) // 16],
                        cap,
                        cap,
                        c,
                    )
                    # selector: sel[p, k, j] = (j == dloc[p, k]) * norm[p, k]
                    sel = spool.tile([128, kt * 128], f16, tag="sel")
                    selv = sel[:].rearrange("p (k j) -> p k j", j=128)
                    iob = iota_sb[:].unsqueeze(1).broadcast_to([128, kt, 128])
                    dlb = dloc_sb[:, base // 128:base // 128 + kt] \
                        .unsqueeze(2).broadcast_to([128, kt, 128])
                    nmb = norm_sb[:, base // 128:base // 128 + kt] \
                        .unsqueeze(2).broadcast_to([128, kt, 128])
                    nc.vector.tensor_tensor(selv, iob, dlb,
                                            mybir.AluOpType.is_equal)
                    nc.vector.tensor_tensor(selv, selv, nmb,
                                            mybir.AluOpType.mult)
                    gtv = gt[:].rearrange("p (k c) -> p k c", c=c)
                    for k in range(kt):
                        nc.tensor.matmul(
                            pa[:],
                            gtv[:, k, :],
                            selv[:, k, :],
                            start=(nmm == 0),
                            stop=(nmm == total_mm - 1),
                        )
                        nmm += 1

                aggT = opool.tile([128, 128], f16, tag="aggT")
                nc.vector.tensor_copy(aggT[:], pa[:])
                po = pout.tile([128, f], f32)
                nc.tensor.matmul(po[:], aggT[:], w_sb[:], start=True, stop=False)
                nc.tensor.matmul(po[:], srow_sb[0:1, d * 128:(d + 1) * 128],
                                 b_sb[:], start=False, stop=True)
                ot = opool.tile([128, f], f16, tag="ot")
                nc.vector.tensor_copy(ot[:], po[:])
                nc.sync.dma_start(out_d[d * 128:(d + 1) * 128, :], ot[:])

    nc.compile()
    return nc


# ---------------------------------------------------------------------------
# host fallback
# ---------------------------------------------------------------------------

def _host_kernel(x, edge_index, W, b):
    row = np.asarray(edge_index[0], dtype=np.int64)
    col = np.asarray(edge_index[1], dtype=np.int64)
    keep = row != col
    deg = np.bincount(row[keep], minlength=N).astype(np.float32) + 1.0
    dis = deg ** -0.5
    xm = np.asarray(x, dtype=np.float32).mean(axis=1)
    from scipy import sparse
    r = row[keep]
    c = col[keep]
    w = (dis[r] * dis[c]).astype(np.float32)
    loops = np.arange(N, dtype=np.int64)
    rows_all = np.concatenate([r, loops])
    cols_all = np.concatenate([c, loops])
    w_all = np.concatenate([w, (1.0 / deg).astype(np.float32)])
    A = sparse.csr_matrix((w_all, (cols_all, rows_all)), shape=(N, N),
                          dtype=np.float32)
    agg = A @ xm
    s = np.asarray(A.sum(axis=1)).ravel().astype(np.float32)
    out = agg @ np.asarray(W, np.float32) + s[:, None] * np.asarray(b, np.float32)[None, :]
    return out.astype(np.float32)


# ---------------------------------------------------------------------------
# device driver
# ---------------------------------------------------------------------------

def _install_ntff_hook():
    """Register the axon NTFF profile hook that the agent image omits."""
    import sys
    try:
        from antenv.axon_hooks import get_axon_ntff_profile_hook  # noqa: F401
        return
    except ImportError:
        pass
    import types
    import antenv
    mod = types.ModuleType("antenv.axon_hooks")
    _state = {"hook": None}
    mod.set_axon_ntff_profile_hook = lambda h: _state.__setitem__("hook", h)
    mod.get_axon_ntff_profile_hook = lambda: _state["hook"]
    sys.modules["antenv.axon_hooks"] = mod
    antenv.axon_hooks = mod
    try:
        from trn_agent_boot.trn_boot import _ntff_profile_via_ctypes
        mod.set_axon_ntff_profile_hook(
            _ntff_profile_via_ctypes("/opt/axon/libaxon_pjrt.so"))
    except Exception:
        pass


def _device_kernel(x, edge_index, W, b):
    global LAST_EXEC_NS, LAST_PROFILE
    import concourse.bass_utils as bass_utils
    if TRACE:
        _install_ntff_hook()

    cfg = CFG
    caps, gidx, dl, nm, srow, _s, _deg, _dis = _preprocess(cfg, edge_index)
    nc = _build_program(cfg, caps)

    x = np.asarray(x)
    xpad = np.zeros((cfg.NCORES, cfg.NPN, cfg.L * cfg.C), F16)
    xf = x.reshape(cfg.N, cfg.L * cfg.C)
    for k in range(cfg.NCORES):
        xpad[k, :cfg.NPC] = xf[k * cfg.NPC:(k + 1) * cfg.NPC].astype(F16)

    wm = (np.asarray(W, np.float32) / cfg.L).astype(F16)
    bv = np.asarray(b, np.float32).astype(F16)[None, :]
    iota = np.tile(np.arange(128, dtype=np.float32)[None, :], (128, 1)).astype(F16)

    in_maps = []
    for k in range(cfg.NCORES):
        in_maps.append({
            "x": xpad[k],
            "gidx": gidx[k],
            "dloc": dl[k],
            "normv": nm[k],
            "srow": srow[k][None, :],
            "wmat": wm,
            "bvec": bv,
            "iota2": iota,
        })

    res = bass_utils.run_bass_kernel_spmd(
        nc, in_maps, core_ids=list(range(cfg.NCORES)), trace=TRACE,
    )
    LAST_EXEC_NS = res.exec_time_ns
    LAST_PROFILE = res.profile_json

    outs = [np.asarray(res.results[k]["out"][:cfg.NPC], np.float32)
            for k in range(cfg.NCORES)]
    return np.concatenate(outs, axis=0)


def kernel(x, edge_index, W, b):
    if not FORCE_HOST:
        try:
            return _device_kernel(x, edge_index, W, b)
        except Exception:
            import traceback
            traceback.print_exc()
    return _host_kernel(x, edge_index, W, b)
